# revision 43
# baseline (speedup 1.0000x reference)
"""Trainium2 Bass kernel for nn_DiffractiveLayer_pixel.

One merged SPMD launch over 8 NeuronCores (default, _mode="merged"):
  - pixel-sharded gumbel-argmax LUT stage (5000 px/core) on GPSIMD/DVE/ACT:
    z = v + g (host-precomputed bit-exact threefry gumbel noise, key 42),
    row-max, exact is_equal mask, masked-sum -> T[argmax] exactly;
  - overlapped batch-sharded DFT chain (2 images/core) on the PE:
    temp^T = G((F w F) o h) G via constant-pair matmuls [Fr|Fi] etc.
    (r/i fused into N=400 matmuls, all lhsT in natural layout, no
    transposes);
  - joined by a 40KB on-device AllGather of the LUT result, then
    cos/sin/amp on ACT, phase multiply, 8x pixel-block expansion via
    0-step broadcast DVE copies, and replicated row writes (0-step source
    DMA) of only the nonzero output rows (outputs are zero-initialized by
    both the native and the axon/PJRT execution paths).

A two-launch fallback (no collectives) is kept under _mode="split".

Self-contained: hardcodes shapes; host precomputes gumbel noise and DFT
constants, reproduces reference f32 semantics exactly (rel err ~3e-7).
"""

import functools
import numpy as np

import concourse.bass as bass
import concourse.tile as tile
from concourse import bacc, mybir
from concourse import bass_utils

F32 = mybir.dt.float32
N_CORES = 8
S = 200
HALF = 100  # chunk size for the 200-dims (2 chunks of 100)
L = 256  # LUT size
B = 16
BPC = B // N_CORES  # batch per core
PIX = S * S
PPC = 5120  # pixels per core (40 tiles of 128); cores overlap slightly
NT = PPC // 128  # 40 LUT tiles per core
TAU_UNUSED = 10.0
AMP = 6.0


# ---------------------------------------------------------------- host consts


@functools.lru_cache(maxsize=1)
def _gumbel():
    """Bit-exact gumbel noise of reference.py (key 42), flat (PIX, L) f32."""
    import jax
    import jax.numpy as jnp

    with jax.default_device(jax.devices("cpu")[0]):
        k1, k2 = jax.random.split(jax.random.key(42))
        u1 = jax.random.uniform(k1, (S, S, L), jnp.float32, 1e-10, 1.0)
        u2 = jax.random.uniform(k2, (S, S, L), jnp.float32, 1e-10, 1.0)
        g1 = -jnp.log(-jnp.log(u1))
        g2 = -jnp.log(-jnp.log(u2))
        g1 = np.ascontiguousarray(np.asarray(g1).reshape(PIX, L))
        g2 = np.ascontiguousarray(np.asarray(g2).reshape(PIX, L))
    return g1, g2


def _chunk2(m):
    """(200, X) -> (100, 2, X): row r=100*kc+p maps to [p, kc, :]."""
    m = np.asarray(m, np.float32)
    return np.ascontiguousarray(m.reshape(2, HALF, m.shape[1]).transpose(1, 0, 2))


@functools.lru_cache(maxsize=1)
def _dft_consts():
    k = np.arange(S)
    ang = (-2.0 * np.pi / S) * np.outer(k, k)
    Fc = np.exp(1j * ang)  # DFT matrix, symmetric
    Gc = np.conj(Fc) / S  # IDFT matrix (applied both sides -> 1/S^2)
    out = {
        "Fr": _chunk2(Fc.real),
        "Fi": _chunk2(Fc.imag),
        "FiN": _chunk2(-Fc.imag),
        "Gr": _chunk2(Gc.real),
        "Gi": _chunk2(Gc.imag),
        "GiN": _chunk2(-Gc.imag),
    }
    e8 = np.zeros((HALF, 800), np.float32)
    for j in range(HALF):
        e8[j, 8 * j : 8 * j + 8] = 1.0
    out["E8"] = e8
    return out


# ------------------------------------------------------------- bass builders


def _build_lut_nc():
    nc = bacc.Bacc("TRN2", target_bir_lowering=False, debug=False)
    v_d = nc.dram_tensor("v", (PPC, L), F32, kind="ExternalInput").ap()
    g1_d = nc.dram_tensor("g1", (PPC, L), F32, kind="ExternalInput").ap()
    g2_d = nc.dram_tensor("g2", (PPC, L), F32, kind="ExternalInput").ap()
    t1_d = nc.dram_tensor("t1", (1, L), F32, kind="ExternalInput").ap()
    t2_d = nc.dram_tensor("t2", (1, L), F32, kind="ExternalInput").ap()
    # output: s[:, 0] = phase_func[argmax1], s[:, 1] = intensity_func[argmax2]
    s_d = nc.dram_tensor("s", (PPC, 2), F32, kind="ExternalOutput").ap()

    GRP = [(0, 10), (10, 10), (20, 10), (30, 10)]  # tile groups per dma

    with tile.TileContext(nc) as tc:
        with (
            tc.tile_pool(name="io", bufs=2) as io,
            tc.tile_pool(name="work", bufs=8) as work,
            tc.tile_pool(name="stat", bufs=1) as stat,
        ):
            t1_sb = stat.tile([128, L], F32)
            t2_sb = stat.tile([128, L], F32)
            nc.sync.dma_start(out=t1_sb, in_=t1_d.to_broadcast((128, L)))
            nc.sync.dma_start(out=t2_sb, in_=t2_d.to_broadcast((128, L)))
            s_sb_all = stat.tile([128, NT, 2], F32)

            v_r = v_d.rearrange("(t p) k -> p t k", p=128)
            g1_r = g1_d.rearrange("(t p) k -> p t k", p=128)
            g2_r = g2_d.rearrange("(t p) k -> p t k", p=128)

            for t0, tn in GRP:
                v_t = io.tile([128, tn, L], F32, tag="v")
                a_t = io.tile([128, tn, L], F32, tag="g1")
                b_t = io.tile([128, tn, L], F32, tag="g2")
                nc.sync.dma_start(out=v_t, in_=v_r[:, t0 : t0 + tn, :])
                nc.sync.dma_start(out=a_t, in_=g1_r[:, t0 : t0 + tn, :])
                nc.sync.dma_start(out=b_t, in_=g2_r[:, t0 : t0 + tn, :])
                for j in range(tn):
                    t = t0 + j
                    for ci, (g_t, t_sb) in enumerate(
                        ((a_t, t1_sb), (b_t, t2_sb))
                    ):
                        z = work.tile([128, L], F32, tag="z")
                        m = work.tile([128, 1], F32, tag="m")
                        nc.gpsimd.tensor_add(z, v_t[:, j, :], g_t[:, j, :])
                        nc.vector.tensor_reduce(
                            out=m, in_=z, axis=mybir.AxisListType.X,
                            op=mybir.AluOpType.max,
                        )
                        mask = work.tile([128, L], F32, tag="mask")
                        # mask = (z == m): exactly 1.0 at argmax, 0.0 else
                        nc.vector.tensor_scalar(
                            out=mask, in0=z, scalar1=m, scalar2=None,
                            op0=mybir.AluOpType.is_equal,
                        )
                        # T[argmax] + 255 exact zeros -> exact gather
                        prod = work.tile([128, L], F32, tag="prod")
                        nc.vector.tensor_mul(prod, mask, t_sb)
                        trash = work.tile([128, L], F32, tag="trash")
                        nc.scalar.activation(
                            out=trash, in_=prod,
                            func=mybir.ActivationFunctionType.Copy,
                            accum_out=s_sb_all[:, t, ci : ci + 1],
                        )

            nc.sync.dma_start(
                out=s_d.rearrange("(t p) c -> p t c", p=128), in_=s_sb_all
            )
    nc.compile()
    return nc


def _build_fft_nc():
    nc = bacc.Bacc("TRN2", target_bir_lowering=False, debug=False)
    dt = lambda n, shp: nc.dram_tensor(n, shp, F32, kind="ExternalInput").ap()
    wr_d = dt("wr", (BPC, S, S))
    wi_d = dt("wi", (BPC, S, S))
    # constants, one tensor: P1=[Fr|Fi] P2=[FiN|Fr] P3=[Gr|Gi] P4=[GiN|Gr]
    # (each (kc, 400)), then hr, hi, ptr, pti (400 each)
    cons_d = dt("cons", (HALF, 4800))
    out_d = nc.dram_tensor(
        "out", (BPC, 10 * S, 10 * S, 2), F32, kind="ExternalOutput"
    ).ap()
    outT = out_d.rearrange("b (i t) w c -> b i t (w c)", t=10)

    with tile.TileContext(nc) as tc:
        with (
            tc.tile_pool(name="cons", bufs=1) as cons,
            tc.tile_pool(name="mats", bufs=2) as mats,
            tc.tile_pool(name="sc", bufs=6) as sc,
            tc.tile_pool(name="xe", bufs=2) as xe,
            tc.tile_pool(name="psum", bufs=6, space="PSUM") as pp,
        ):
            cons_t = cons.tile([HALF, 4800], F32, name="c_all")
            nc.sync.dma_start(out=cons_t, in_=cons_d)
            P = {
                k: cons_t[:, 800 * i : 800 * i + 800].rearrange(
                    "p (kc n) -> p kc n", kc=2
                )
                for i, k in enumerate(("P1", "P2", "P3", "P4"))
            }
            hh = {
                k: cons_t[:, o : o + 400].rearrange("p (kc n) -> p kc n", kc=2)
                for k, o in (
                    ("hr", 3200), ("hi", 3600), ("ptr", 4000), ("pti", 4400)
                )
            }

            for b in range(BPC):
                w_sb = {}
                for k, d in (("wr", wr_d), ("wi", wi_d)):
                    w_sb[k] = mats.tile([HALF, 2, S], F32, tag=k, name=f"w_{k}_{b}")
                    nc.sync.dma_start(
                        out=w_sb[k],
                        in_=d[b].rearrange("(kc p) n -> p kc n", p=HALF),
                    )

                # S1: AT = w^T F; psum [n, (ATr|ATi)] via const rhs pairs
                at = mats.tile([HALF, 2, 2, S], F32, tag="at", name=f"at_{b}")
                for mc in range(2):
                    ps = pp.tile([128, 512], F32, tag="ps", name=f"ps1_{b}")
                    i = 0
                    for kc in range(2):
                        for lh, pr in ((w_sb["wr"], "P1"), (w_sb["wi"], "P2")):
                            nc.tensor.matmul(
                                ps[:HALF, :400],
                                lh[:, kc, mc * HALF : (mc + 1) * HALF],
                                P[pr][:, kc, :],
                                start=(i == 0), stop=(i == 3),
                            )
                            i += 1
                    nc.scalar.copy(
                        at[:, mc, :, :],
                        ps[:HALF, :400].rearrange("p (c n) -> p c n", c=2),
                    )

                # S2: B = A F (fft2); psum [m, (Br|Bi)]; then Y = B o h
                yy = mats.tile([HALF, 2, 2, S], F32, tag="yy", name=f"yy_{b}")
                for mc in range(2):
                    ps = pp.tile([128, 512], F32, tag="ps", name=f"ps2_{b}")
                    i = 0
                    for kc in range(2):
                        for ci, pr in ((0, "P1"), (1, "P2")):
                            nc.tensor.matmul(
                                ps[:HALF, :400],
                                at[:, kc, ci, mc * HALF : (mc + 1) * HALF],
                                P[pr][:, kc, :],
                                start=(i == 0), stop=(i == 3),
                            )
                            i += 1
                    ta = sc.tile([HALF, S], F32, tag="ta")
                    tb = sc.tile([HALF, S], F32, tag="tb")
                    nc.vector.tensor_mul(ta, ps[:HALF, 0:S], hh["hr"][:, mc, :])
                    nc.vector.tensor_mul(tb, ps[:HALF, S:400], hh["hi"][:, mc, :])
                    nc.vector.tensor_sub(yy[:, mc, 0, :], ta, tb)
                    nc.vector.tensor_mul(ta, ps[:HALF, 0:S], hh["hi"][:, mc, :])
                    nc.vector.tensor_mul(tb, ps[:HALF, S:400], hh["hr"][:, mc, :])
                    nc.vector.tensor_add(yy[:, mc, 1, :], ta, tb)

                # S3: T1T = Y^T G; psum [f, (T1Tr|T1Ti)]
                t1t = mats.tile([HALF, 2, 2, S], F32, tag="t1t", name=f"t1t_{b}")
                for fc in range(2):
                    ps = pp.tile([128, 512], F32, tag="ps", name=f"ps3_{b}")
                    i = 0
                    for kc in range(2):
                        for ci, pr in ((0, "P3"), (1, "P4")):
                            nc.tensor.matmul(
                                ps[:HALF, :400],
                                yy[:, kc, ci, fc * HALF : (fc + 1) * HALF],
                                P[pr][:, kc, :],
                                start=(i == 0), stop=(i == 3),
                            )
                            i += 1
                    nc.scalar.copy(
                        t1t[:, fc, 0:2, :],
                        ps[:HALF, :400].rearrange("p (c n) -> p c n", c=2),
                    )

                # S4: X = T1 G, natural [i, j]; lhsT = T1T, rhs = G pairs
                # psum [i, (Xr|Xi)]; then Xs = X o pt
                xs = mats.tile([HALF, 2, 2, S], F32, tag="xs", name=f"xs_{b}")
                for ic in range(2):
                    ps = pp.tile([128, 512], F32, tag="ps", name=f"ps4_{b}")
                    i = 0
                    for kc in range(2):
                        for ci, pr in ((0, "P3"), (1, "P4")):
                            nc.tensor.matmul(
                                ps[:HALF, :400],
                                t1t[:, kc, ci, ic * HALF : (ic + 1) * HALF],
                                P[pr][:, kc, :],
                                start=(i == 0), stop=(i == 3),
                            )
                            i += 1
                    ta = sc.tile([HALF, S], F32, tag="ta")
                    tb = sc.tile([HALF, S], F32, tag="tb")
                    nc.vector.tensor_mul(ta, ps[:HALF, 0:S], hh["ptr"][:, ic, :])
                    nc.vector.tensor_mul(tb, ps[:HALF, S:400], hh["pti"][:, ic, :])
                    nc.vector.tensor_sub(xs[:, ic, 0, :], ta, tb)
                    nc.vector.tensor_mul(ta, ps[:HALF, 0:S], hh["pti"][:, ic, :])
                    nc.vector.tensor_mul(tb, ps[:HALF, S:400], hh["ptr"][:, ic, :])
                    nc.vector.tensor_add(xs[:, ic, 1, :], ta, tb)

                # S5: 8x column replication + r/i interleave via 0-step
                # DVE copies, then replicated row writes
                buf = xe.tile([HALF, 2, 10 * S * 2], F32, tag="xebuf")
                bufv = buf.rearrange("p ic (j d c) -> p ic j d c", d=10, c=2)
                nc.vector.memset(bufv[:, :, :, 0, :], 0.0)
                nc.vector.memset(bufv[:, :, :, 9, :], 0.0)
                for ic in range(2):
                    for half in range(2):
                        jsl = slice(half * HALF, half * HALF + HALF)
                        for ci in range(2):
                            xv = xs[:, ic, ci, jsl]
                            rd = bass.AP(
                                tensor=xv.tensor, offset=xv.offset,
                                ap=[xv.ap[0], xv.ap[1], [0, 8]],
                            )
                            nc.vector.tensor_copy(
                                bufv[:, ic, jsl, 1:9, ci], rd
                            )
                    for nck in range(4):
                        v = buf[:, ic, 1000 * nck : 1000 * nck + 1000]
                        rep = bass.AP(
                            tensor=v.tensor, offset=v.offset,
                            ap=[v.ap[0], [0, 8], v.ap[1]],
                        )
                        nc.sync.dma_start(
                            out=outT[
                                b,
                                ic * HALF : ic * HALF + HALF,
                                1:9,
                                1000 * nck : 1000 * nck + 1000,
                            ],
                            in_=rep,
                        )
    nc.compile()
    return nc


def _build_merged_nc(timing_proxy=False):
    """Single launch: pixel-sharded LUT (POOL/DVE/ACT) overlapped with the
    batch-sharded DFT chain (PE), joined by a 40KB AllGather of the LUT
    result; then phase multiply, expansion, replicated row writes.

    timing_proxy=True replaces the collective with a local DMA so the
    (single-core, collective-free) TimelineSim cost model can run."""
    PPX = PIX // N_CORES  # 5000 pixels per core, exact
    FULL = PPX // 128  # 39 full LUT tiles
    REM = PPX - FULL * 128  # 8 leftover pixels
    nc = bacc.Bacc("TRN2", target_bir_lowering=False, debug=False, num_devices=N_CORES)
    dt = lambda n, shp: nc.dram_tensor(n, shp, F32, kind="ExternalInput").ap()
    v_d = dt("v", (PPX, L))
    g1_d = dt("g1", (PPX, L))
    g2_d = dt("g2", (PPX, L))
    t1_d = dt("t1", (1, L))
    t2_d = dt("t2", (1, L))
    wr_d = dt("wr", (BPC, S, S))
    wi_d = dt("wi", (BPC, S, S))
    # P1=[Fr|Fi] P2=[FiN|Fr] P3=[Gr|Gi] P4=[GiN|Gr] (kc, 400 each), hr, hi
    cons_d = dt("cons", (HALF, 4000))
    out_d = nc.dram_tensor(
        "out", (BPC, 10 * S, 10 * S, 2), F32, kind="ExternalOutput"
    ).ap()
    outT = out_d.rearrange("b (i t) w c -> b i t (w c)", t=10)
    cc_in = nc.dram_tensor("cc_in", (PPX, 2), F32, kind="Internal").ap()
    cc_out = nc.dram_tensor(
        "cc_out", (N_CORES, PPX, 2), F32, kind="Internal", addr_space="Shared"
    ).ap()

    with tile.TileContext(nc) as tc:
        with (
            tc.tile_pool(name="cons", bufs=1) as cons,
            tc.tile_pool(name="io", bufs=3) as io,
            tc.tile_pool(name="work", bufs=6) as work,
            tc.tile_pool(name="mats", bufs=2) as mats,
            tc.tile_pool(name="sc", bufs=6) as sc,
            tc.tile_pool(name="xe", bufs=2) as xe,
            tc.tile_pool(name="psum", bufs=6, space="PSUM") as pp,
        ):
            # FFT-phase inputs FIRST so PE work is not queued behind the
            # 15.7MB of LUT loads on the sync DMA ring
            cons_t = cons.tile([HALF, 4000], F32, name="c_all")
            nc.sync.dma_start(out=cons_t, in_=cons_d)
            w_all = {}
            for b in range(BPC):
                for k, d in (("wr", wr_d), ("wi", wi_d)):
                    w_all[k, b] = mats.tile(
                        [HALF, 2, S], F32, tag=k, name=f"w_{k}_{b}"
                    )
                    nc.sync.dma_start(
                        out=w_all[k, b],
                        in_=d[b].rearrange("(kc p) n -> p kc n", p=HALF),
                    )

            # warm the ACT Sin function-set table before it is needed on
            # the post-collective critical path
            warm = cons.tile([1, 1], F32)
            nc.vector.memset(warm, 0.0)
            warm2 = cons.tile([1, 1], F32)
            nc.scalar.activation(
                out=warm2, in_=warm,
                func=mybir.ActivationFunctionType.Sin, bias=warm,
            )

            # ---------------- phase A: LUT argmax gather on 1/8 of pixels
            t1_sb = cons.tile([128, L], F32)
            t2_sb = cons.tile([128, L], F32)
            nc.sync.dma_start(out=t1_sb, in_=t1_d.to_broadcast((128, L)))
            nc.sync.dma_start(out=t2_sb, in_=t2_d.to_broadcast((128, L)))
            s_sb = cons.tile([128, FULL + 1, 2], F32)

            vf = v_d[: FULL * 128].rearrange("(t p) k -> p t k", p=128)
            g1f = g1_d[: FULL * 128].rearrange("(t p) k -> p t k", p=128)
            g2f = g2_d[: FULL * 128].rearrange("(t p) k -> p t k", p=128)

            def lut_tile(t, vt, at, bt, j, pn):
                for ci, (g_t, t_sb) in enumerate(((at, t1_sb), (bt, t2_sb))):
                    # ~1/6 of the (iseq, mult) pairs go to the otherwise
                    # idle POOL engine to shorten the DVE-bound LUT span
                    alt = nc.gpsimd if (ci == 1 and t % 3 == 2) else nc.vector
                    z = work.tile([128, L], F32, tag="z")
                    m = work.tile([128, 1], F32, tag="m")
                    nc.gpsimd.tensor_add(
                        z[:pn], vt[:pn, j, :], g_t[:pn, j, :]
                    )
                    nc.vector.tensor_reduce(
                        out=m[:pn], in_=z[:pn],
                        axis=mybir.AxisListType.X, op=mybir.AluOpType.max,
                    )
                    mask = work.tile([128, L], F32, tag="mask")
                    alt.tensor_scalar(
                        out=mask[:pn], in0=z[:pn], scalar1=m[:pn],
                        scalar2=None, op0=mybir.AluOpType.is_equal,
                    )
                    prod = work.tile([128, L], F32, tag="prod")
                    alt.tensor_mul(prod[:pn], mask[:pn], t_sb[:pn])
                    nc.scalar.activation(
                        out=prod[:pn], in_=prod[:pn],
                        func=mybir.ActivationFunctionType.Copy,
                        accum_out=s_sb[:pn, t, ci : ci + 1],
                    )

            for t0, tn in ((0, 5), (5, 5), (10, 5), (15, 5), (20, 5),
                           (25, 5), (30, 5), (35, 4)):
                v_t = io.tile([128, 5, L], F32, tag="v")
                a_t = io.tile([128, 5, L], F32, tag="g1")
                b_t = io.tile([128, 5, L], F32, tag="g2")
                nc.sync.dma_start(out=v_t[:, :tn], in_=vf[:, t0 : t0 + tn, :])
                nc.sync.dma_start(out=a_t[:, :tn], in_=g1f[:, t0 : t0 + tn, :])
                nc.sync.dma_start(out=b_t[:, :tn], in_=g2f[:, t0 : t0 + tn, :])
                for j in range(tn):
                    lut_tile(t0 + j, v_t, a_t, b_t, j, 128)
            # leftover 8 pixels
            v_p = io.tile([REM, L], F32, tag="vp")
            a_p = io.tile([REM, L], F32, tag="g1p")
            b_p = io.tile([REM, L], F32, tag="g2p")
            nc.sync.dma_start(out=v_p, in_=v_d[FULL * 128 :])
            nc.sync.dma_start(out=a_p, in_=g1_d[FULL * 128 :])
            nc.sync.dma_start(out=b_p, in_=g2_d[FULL * 128 :])
            lut_tile(FULL, v_p[:, None, :], a_p[:, None, :], b_p[:, None, :], 0, REM)

            nc.sync.dma_start(
                out=cc_in[: FULL * 128].rearrange("(t p) c -> p t c", p=128),
                in_=s_sb[:, :FULL, :],
            )
            nc.sync.dma_start(out=cc_in[FULL * 128 :], in_=s_sb[:REM, FULL, :])
            if timing_proxy:
                for c in range(N_CORES):
                    nc.sync.dma_start(out=cc_out[c], in_=cc_in)
            else:
                nc.gpsimd.collective_compute(
                    "AllGather", mybir.AluOpType.bypass,
                    ins=[cc_in], outs=[cc_out],
                    replica_groups=[list(range(N_CORES))],
                )

            # gathered s -> pt tiles [100, ic, 200] (pixel p = 200 i + j)
            st = cons.tile([HALF, 2, S, 2], F32)
            cc_flat = cc_out.rearrange("c q x -> (c q) x").rearrange(
                "(ic p j) x -> p ic j x", ic=2, p=HALF
            )
            for icq in range(2):
                nc.sync.dma_start(
                    out=st[:, icq : icq + 1], in_=cc_flat[:, icq : icq + 1]
                )
            ptr_sb = cons.tile([HALF, 2, S], F32)
            pti_sb = cons.tile([HALF, 2, S], F32)
            cosv = cons.tile([HALF, 2, S], F32)
            sinv = cons.tile([HALF, 2, S], F32)
            ampv = cons.tile([HALF, 2, S], F32)
            bias_h = cons.tile([HALF, 1], F32)
            nc.vector.memset(bias_h, float(np.pi / 2))
            bias_z = cons.tile([HALF, 1], F32)
            nc.vector.memset(bias_z, 0.0)
            for icq in range(2):
                nc.scalar.activation(
                    out=cosv[:, icq], in_=st[:, icq, :, 0],
                    func=mybir.ActivationFunctionType.Sin,
                    bias=bias_h, scale=-1.0,
                )
                nc.scalar.activation(
                    out=sinv[:, icq], in_=st[:, icq, :, 0],
                    func=mybir.ActivationFunctionType.Sin,
                    bias=bias_z,
                )
                nc.vector.tensor_scalar(
                    out=ampv[:, icq], in0=st[:, icq, :, 1], scalar1=AMP,
                    scalar2=None, op0=mybir.AluOpType.mult,
                )
                nc.vector.tensor_mul(ptr_sb[:, icq], ampv[:, icq], cosv[:, icq])
                nc.vector.tensor_mul(pti_sb[:, icq], ampv[:, icq], sinv[:, icq])

            # ---------------- phase B: DFT chain + expansion per batch item
            P = {
                k: cons_t[:, 800 * i : 800 * i + 800].rearrange(
                    "p (kc n) -> p kc n", kc=2
                )
                for i, k in enumerate(("P1", "P2", "P3", "P4"))
            }
            hh = {
                "hr": cons_t[:, 3200:3600].rearrange("p (kc n) -> p kc n", kc=2),
                "hi": cons_t[:, 3600:4000].rearrange("p (kc n) -> p kc n", kc=2),
                "ptr": ptr_sb,
                "pti": pti_sb,
            }

            for b in range(BPC):
                w_sb = {k: w_all[k, b] for k in ("wr", "wi")}

                at = mats.tile([HALF, 2, 2, S], F32, tag="at", name=f"at_{b}")
                for mc in range(2):
                    ps = pp.tile([128, 512], F32, tag="ps", name=f"ps1_{b}")
                    i = 0
                    for kc in range(2):
                        for lh, pr in ((w_sb["wr"], "P1"), (w_sb["wi"], "P2")):
                            nc.tensor.matmul(
                                ps[:HALF, :400],
                                lh[:, kc, mc * HALF : (mc + 1) * HALF],
                                P[pr][:, kc, :],
                                start=(i == 0), stop=(i == 3),
                            )
                            i += 1
                    nc.scalar.copy(
                        at[:, mc, :, :],
                        ps[:HALF, :400].rearrange("p (c n) -> p c n", c=2),
                    )

                yy = mats.tile([HALF, 2, 2, S], F32, tag="yy", name=f"yy_{b}")
                for mc in range(2):
                    ps = pp.tile([128, 512], F32, tag="ps", name=f"ps2_{b}")
                    i = 0
                    for kc in range(2):
                        for ci, pr in ((0, "P1"), (1, "P2")):
                            nc.tensor.matmul(
                                ps[:HALF, :400],
                                at[:, kc, ci, mc * HALF : (mc + 1) * HALF],
                                P[pr][:, kc, :],
                                start=(i == 0), stop=(i == 3),
                            )
                            i += 1
                    ta = sc.tile([HALF, S], F32, tag="ta")
                    tb = sc.tile([HALF, S], F32, tag="tb")
                    nc.vector.tensor_mul(ta, ps[:HALF, 0:S], hh["hr"][:, mc, :])
                    nc.vector.tensor_mul(tb, ps[:HALF, S:400], hh["hi"][:, mc, :])
                    nc.vector.tensor_sub(yy[:, mc, 0, :], ta, tb)
                    nc.vector.tensor_mul(ta, ps[:HALF, 0:S], hh["hi"][:, mc, :])
                    nc.vector.tensor_mul(tb, ps[:HALF, S:400], hh["hr"][:, mc, :])
                    nc.vector.tensor_add(yy[:, mc, 1, :], ta, tb)

                t1t = mats.tile([HALF, 2, 2, S], F32, tag="t1t", name=f"t1t_{b}")
                for fc in range(2):
                    ps = pp.tile([128, 512], F32, tag="ps", name=f"ps3_{b}")
                    i = 0
                    for kc in range(2):
                        for ci, pr in ((0, "P3"), (1, "P4")):
                            nc.tensor.matmul(
                                ps[:HALF, :400],
                                yy[:, kc, ci, fc * HALF : (fc + 1) * HALF],
                                P[pr][:, kc, :],
                                start=(i == 0), stop=(i == 3),
                            )
                            i += 1
                    nc.scalar.copy(
                        t1t[:, fc, 0:2, :],
                        ps[:HALF, :400].rearrange("p (c n) -> p c n", c=2),
                    )

                xs = mats.tile([HALF, 2, 2, S], F32, tag="xs", name=f"xs_{b}")
                for ic in range(2):
                    ps = pp.tile([128, 512], F32, tag="ps", name=f"ps4_{b}")
                    i = 0
                    for kc in range(2):
                        for ci, pr in ((0, "P3"), (1, "P4")):
                            nc.tensor.matmul(
                                ps[:HALF, :400],
                                t1t[:, kc, ci, ic * HALF : (ic + 1) * HALF],
                                P[pr][:, kc, :],
                                start=(i == 0), stop=(i == 3),
                            )
                            i += 1
                    ta = sc.tile([HALF, S], F32, tag="ta")
                    tb = sc.tile([HALF, S], F32, tag="tb")
                    nc.vector.tensor_mul(ta, ps[:HALF, 0:S], hh["ptr"][:, ic, :])
                    nc.vector.tensor_mul(tb, ps[:HALF, S:400], hh["pti"][:, ic, :])
                    nc.vector.tensor_sub(xs[:, ic, 0, :], ta, tb)
                    nc.vector.tensor_mul(ta, ps[:HALF, 0:S], hh["pti"][:, ic, :])
                    nc.vector.tensor_mul(tb, ps[:HALF, S:400], hh["ptr"][:, ic, :])
                    nc.vector.tensor_add(xs[:, ic, 1, :], ta, tb)

                buf = xe.tile([HALF, 2, 10 * S * 2], F32, tag="xebuf")
                bufv = buf.rearrange("p ic (j d c) -> p ic j d c", d=10, c=2)
                nc.vector.memset(bufv[:, :, :, 0, :], 0.0)
                nc.vector.memset(bufv[:, :, :, 9, :], 0.0)
                for ic in range(2):
                    for half in range(2):
                        jsl = slice(half * HALF, half * HALF + HALF)
                        for ci in range(2):
                            xv = xs[:, ic, ci, jsl]
                            rd = bass.AP(
                                tensor=xv.tensor, offset=xv.offset,
                                ap=[xv.ap[0], xv.ap[1], [0, 8]],
                            )
                            nc.vector.tensor_copy(bufv[:, ic, jsl, 1:9, ci], rd)
                    v = buf[:, ic, :]
                    rep = bass.AP(
                        tensor=v.tensor, offset=v.offset,
                        ap=[v.ap[0], [0, 8], v.ap[1]],
                    )
                    nc.sync.dma_start(
                        out=outT[b, ic * HALF : ic * HALF + HALF, 1:9, :],
                        in_=rep,
                    )
    nc.compile()
    return nc


_NC_CACHE = {}


def _get_nc(name):
    if name not in _NC_CACHE:
        _NC_CACHE[name] = {
            "lut": _build_lut_nc,
            "fft": _build_fft_nc,
            "merged": _build_merged_nc,
        }[name]()
    return _NC_CACHE[name]


def _kernel_merged(waves_real, waves_imag, h_real, h_imag, voltage,
                   phase_func, intensity_func, _results=None):
    g1, g2 = _gumbel()
    v_flat = voltage.reshape(PIX, L)
    t1 = phase_func.reshape(1, L)
    t2 = intensity_func.reshape(1, L)
    dc = _dft_consts()
    pair = lambda a, b: np.concatenate([dc[a], dc[b]], axis=2).reshape(HALF, 800)
    cons = np.concatenate(
        [
            pair("Fr", "Fi"), pair("FiN", "Fr"),
            pair("Gr", "Gi"), pair("GiN", "Gr"),
            _chunk2(np.asarray(h_real, np.float32)).reshape(HALF, 400),
            _chunk2(np.asarray(h_imag, np.float32)).reshape(HALF, 400),
        ],
        axis=1,
    )
    PPX = PIX // N_CORES
    in_maps = [
        {
            "v": v_flat[PPX * c : PPX * (c + 1)],
            "g1": g1[PPX * c : PPX * (c + 1)],
            "g2": g2[PPX * c : PPX * (c + 1)],
            "t1": t1,
            "t2": t2,
            "wr": waves_real[BPC * c : BPC * (c + 1)],
            "wi": waves_imag[BPC * c : BPC * (c + 1)],
            "cons": cons,
        }
        for c in range(N_CORES)
    ]
    nc = _get_nc("merged")
    res = bass_utils.run_bass_kernel_spmd(nc, in_maps, core_ids=list(range(N_CORES)))
    if _results is not None:
        _results.append(res)
    return np.concatenate([r["out"] for r in res.results], axis=0)


# --------------------------------------------------------------------- kernel


def kernel(
    waves_real,
    waves_imag,
    h_real,
    h_imag,
    voltage,
    phase_func,
    intensity_func,
    _results=None,
    _mode="merged",
):
    waves_real = np.ascontiguousarray(np.asarray(waves_real, np.float32))
    waves_imag = np.ascontiguousarray(np.asarray(waves_imag, np.float32))
    voltage = np.ascontiguousarray(np.asarray(voltage, np.float32))
    phase_func = np.asarray(phase_func, np.float32)
    intensity_func = np.asarray(intensity_func, np.float32)

    if _mode == "merged":
        return _kernel_merged(
            waves_real, waves_imag, h_real, h_imag, voltage,
            phase_func, intensity_func, _results=_results,
        )

    g1, g2 = _gumbel()
    v_flat = voltage.reshape(PIX, L)
    t1 = phase_func.reshape(1, L)
    t2 = intensity_func.reshape(1, L)

    offs = [min(PPC * c, PIX - PPC) for c in range(N_CORES)]
    in_maps1 = [
        {
            "v": v_flat[o : o + PPC],
            "g1": g1[o : o + PPC],
            "g2": g2[o : o + PPC],
            "t1": t1,
            "t2": t2,
        }
        for o in offs
    ]
    nc1 = _get_nc("lut")
    res1 = bass_utils.run_bass_kernel_spmd(
        nc1, in_maps1, core_ids=list(range(N_CORES))
    )
    s_flat = np.empty((PIX, 2), np.float32)
    for c, o in enumerate(offs):
        # core 7's range overlaps core 6's; identical values, overwrite is fine
        s_flat[o : o + PPC] = res1.results[c]["s"]

    # host: phase_trig = (6*intensity[am2]) * (cos + i sin)(phase[am1]),
    # mirroring reference f32 ops
    theta = s_flat[:, 0]
    amp = (s_flat[:, 1] * np.float32(AMP)).astype(np.float32)
    pt_r = (amp * np.cos(theta)).astype(np.float32)
    pt_i = (amp * np.sin(theta)).astype(np.float32)
    ptr = pt_r.reshape(S, S)
    pti = pt_i.reshape(S, S)

    dc = _dft_consts()
    pair = lambda a, b: np.concatenate([dc[a], dc[b]], axis=2).reshape(HALF, 800)
    cons = np.concatenate(
        [
            pair("Fr", "Fi"), pair("FiN", "Fr"),
            pair("Gr", "Gi"), pair("GiN", "Gr"),
            _chunk2(np.asarray(h_real, np.float32)).reshape(HALF, 400),
            _chunk2(np.asarray(h_imag, np.float32)).reshape(HALF, 400),
            _chunk2(ptr).reshape(HALF, 400),
            _chunk2(pti).reshape(HALF, 400),
        ],
        axis=1,
    )
    in_maps2 = [
        {
            "wr": waves_real[BPC * c : BPC * (c + 1)],
            "wi": waves_imag[BPC * c : BPC * (c + 1)],
            "cons": cons,
        }
        for c in range(N_CORES)
    ]
    nc2 = _get_nc("fft")
    res2 = bass_utils.run_bass_kernel_spmd(
        nc2, in_maps2, core_ids=list(range(N_CORES))
    )
    if _results is not None:
        _results.extend([res1, res2])
    return np.concatenate([r["out"] for r in res2.results], axis=0)
